# revision 1
# baseline (speedup 1.0000x reference)
# Multi-head attention (B=2, S=4096, D=512, H=8) on 8 Trainium2 NeuronCores.
#
# Sharding: core c handles batch b=c//4 and query rows [(c%4)*1024, (c%4+1)*1024).
# Each core computes K/V projections for its full batch element (duplicated
# across the 4 cores sharing a batch; avoids all cross-core communication),
# Q for its own slice, full 8-head attention for its query rows, and the
# output projection for its rows. The full output is the disjoint
# concatenation of the 8 per-core results.
#
# On-core dataflow (all matmuls bf16 with fp32 PSUM accumulation):
#   x,W --SWDGE cast--> bf16 DRAM --DMA-transpose--> x^T, W^T in SBUF
#   Q^T = Wq^T-tiles x x^T   K^T = Wk^T-tiles x x^T   V = x^T-tiles x Wv^T
#   per (q-chunk 512, head-pair): for each k-tile of 128:
#     S^T[k,q] = K^T x Q^T     (two heads row-packed in the PE array)
#     P^T = exp(S^T * scale)   (ACT engine, PSUM->SBUF, scale via free affine)
#     O^T[dh,q] += V-tile^T x P^T ; l[q] += ones^T x P^T  (column-packed)
#   O^T normalized by 1/l (DVE; l is produced broadcast across partitions)
#   y = O^T-tiles x Wout^T + b  (consumes O^T directly as stationary operand)
import sys

if "/opt/trn_rl_repo" not in sys.path:
    sys.path.insert(0, "/opt/trn_rl_repo")

import numpy as np

B = 2
S = 4096
DIM = 512
H = 8
DH = DIM // H
SCALE = DH**-0.5
N_CORES = 8
QLOC = S // 4  # query rows per core
N_KT = S // 128  # k tiles of 128
N_DT = DIM // 128  # feature-dim tiles of 128

_CACHE = {}


def _build_program(reps=1):
    import os
    from contextlib import ExitStack

    from concourse import bacc, mybir, tile

    ablate = set(os.environ.get("BENCH_ABLATE", "").split(","))

    f32 = mybir.dt.float32
    bf16 = mybir.dt.bfloat16
    Exp = mybir.ActivationFunctionType.Exp

    nc = bacc.Bacc("TRN2", target_bir_lowering=False, debug=False)

    x_full = nc.dram_tensor("x_full", [S, DIM], f32, kind="ExternalInput")
    x_q = nc.dram_tensor("x_q", [QLOC, DIM], f32, kind="ExternalInput")
    w_qkv = nc.dram_tensor("w_qkv", [3 * DIM, DIM], f32, kind="ExternalInput")
    w_out = nc.dram_tensor("w_out", [DIM, DIM], f32, kind="ExternalInput")
    b_out = nc.dram_tensor("b_out", [1, DIM], f32, kind="ExternalInput")
    y_out = nc.dram_tensor("y", [QLOC, DIM], f32, kind="ExternalOutput")

    x_bf = nc.dram_tensor("x_bf", [S, DIM], bf16)
    xq_bf = nc.dram_tensor("xq_bf", [QLOC, DIM], bf16)
    wqkv_bf = nc.dram_tensor("wqkv_bf", [3 * DIM, DIM], bf16)
    wout_bf = nc.dram_tensor("wout_bf", [DIM, DIM], bf16)

    with tile.TileContext(nc) as tc, ExitStack() as ctx:
        if reps > 1:  # benchmarking only: repeat the whole body in a HW loop
            ctx.enter_context(tc.For_i(0, reps, 1))
        consts = ctx.enter_context(tc.tile_pool(name="consts", bufs=1))
        wp = ctx.enter_context(tc.tile_pool(name="wp", bufs=1))
        big = ctx.enter_context(tc.tile_pool(name="big", bufs=1))
        xtp = ctx.enter_context(tc.tile_pool(name="xtp", bufs=2))
        ptp = ctx.enter_context(tc.tile_pool(name="ptp", bufs=6))
        otp = ctx.enter_context(tc.tile_pool(name="otp", bufs=2))
        rbp = ctx.enter_context(tc.tile_pool(name="rbp", bufs=2))
        ysp = ctx.enter_context(tc.tile_pool(name="ysp", bufs=2))
        pp = ctx.enter_context(tc.tile_pool(name="pp", bufs=1, space="PSUM"))
        sp = ctx.enter_context(tc.tile_pool(name="sp", bufs=1, space="PSUM"))
        op = ctx.enter_context(tc.tile_pool(name="op", bufs=1, space="PSUM"))

        # --- constants ---
        ones_sb = consts.tile([128, 64], bf16, tag="ones")
        nc.gpsimd.memset(ones_sb[:], 1.0)
        # zeros: lhsT/rhs of the accumulation-group bracket matmuls (see below)
        zeros_sb = consts.tile([128, 512], bf16, tag="zeros")
        nc.gpsimd.memset(zeros_sb[:], 0.0)
        bias_sb = consts.tile([128, DIM], f32, tag="bias")
        nc.gpsimd.dma_start(out=bias_sb[:], in_=b_out.ap().broadcast_to([128, DIM]))

        # --- f32 -> bf16 casts: HWDGE load -> DVE cast -> HWDGE store ---
        # (SWDGE DRAM->DRAM cast DMAs serialize on the GPSIMD engine; this
        # path keeps the cast on the otherwise-idle DVE + fast HWDGE queues.)
        castp = ctx.enter_context(tc.tile_pool(name="castp", bufs=2))
        stgp = ctx.enter_context(tc.tile_pool(name="stgp", bufs=6))
        cast_n = [0]

        def cast_chunk(dst_dram, src_dram, r0, rows):
            a = rows // 128
            cast_n[0] += 1
            xf = castp.tile([128, a, DIM], f32, tag="castf", name=f"castf{cast_n[0]}")
            nc.sync.dma_start(
                out=xf[:],
                in_=src_dram.ap()[r0 : r0 + rows, :].rearrange(
                    "(a p) d -> p a d", p=128
                ),
            )
            xb = castp.tile([128, a, DIM], bf16, tag="castb", name=f"castb{cast_n[0]}")
            nc.gpsimd.tensor_copy(xb[:], xf[:])
            nc.sync.dma_start(
                out=dst_dram.ap()[r0 : r0 + rows, :].rearrange(
                    "(a p) d -> p a d", p=128
                ),
                in_=xb[:],
            )

        for ec in range(3):
            cast_chunk(wqkv_bf, w_qkv, ec * 512, 512)
        cast_chunk(wout_bf, w_out, 0, 512)
        for qc in range(QLOC // 512):
            cast_chunk(xq_bf, x_q, qc * 512, 512)

        # --- W^T via DMA transpose: wqkvT[d, e] layout [128, dt, 1536] ---
        wqkvT = wp.tile([128, N_DT, 3 * DIM], bf16, tag="wqkvT")
        for dt in range(N_DT):
            for ec in range(3):
                nc.sync.dma_start(
                    out=wqkvT[:, dt, ec * 512 : (ec + 1) * 512],
                    in_=wqkv_bf.ap()[ec * 512 : (ec + 1) * 512, dt * 128 : (dt + 1) * 128],
                    transpose=True,
                )
        woutT = wp.tile([128, N_DT, DIM], bf16, tag="woutT")
        for dt in range(N_DT):
            nc.sync.dma_start(
                out=woutT[:, dt, :],
                in_=wout_bf.ap()[:, dt * 128 : (dt + 1) * 128],
                transpose=True,
            )

        # --- persistent per-core tensors ---
        KT = big.tile([128, N_DT, S], bf16, tag="KT")  # K^T: part=(e-512)%128, [et, s]
        V = big.tile([128, S // 128, DIM], bf16, tag="V")  # V: part=s%128, [s-tile, e]
        QT = big.tile([128, N_DT, QLOC], bf16, tag="QT")  # Q^T: part=e%128, [et, q]

        if "noproj" in ablate:
            nc.gpsimd.memset(KT[:], 0.25)
            nc.gpsimd.memset(V[:], 0.25)
            nc.gpsimd.memset(QT[:], 0.25)

        # --- Q^T projection (own query slice) ---
        for qc in range(QLOC // 512) if "noproj" not in ablate else []:
            xqT = xtp.tile([128, N_DT, 512], bf16, tag="xT")
            for dt in range(N_DT):
                nc.sync.dma_start(
                    out=xqT[:, dt, :],
                    in_=xq_bf.ap()[qc * 512 : (qc + 1) * 512, dt * 128 : (dt + 1) * 128],
                    transpose=True,
                )
            for et in range(N_DT):
                ps = pp.tile([128, 512], f32, tag="proj")
                for dt in range(N_DT):
                    nc.tensor.matmul(
                        ps[:],
                        wqkvT[:, dt, et * 128 : (et + 1) * 128],
                        xqT[:, dt, :],
                        start=(dt == 0),
                        stop=(dt == N_DT - 1),
                    )
                nc.vector.tensor_copy(QT[:, et, qc * 512 : (qc + 1) * 512], ps[:])

        # --- attention pair machinery ---
        ot_state = {}
        stg_state = {}

        def pair_begin(qc, j):
            if "probe" in ablate:
                ot_state[(qc, j)] = (None, None)
                return
            # The two packed heads accumulate into disjoint partition halves of
            # one PSUM bank. Hardware tracks has_written per element, but the
            # start/stop accumulation-group flags act on the whole bank region,
            # so open each bank with a single full-128-partition zeroing matmul
            # (and close it symmetrically in pair_end) — exactly one
            # accumulation group per bank.
            ot = op.tile([128, 512], f32, tag="ot", bufs=2)
            lt = op.tile([128, 512], f32, tag="lt", bufs=1)
            for t in (ot, lt):
                nc.tensor.matmul(
                    t[:], zeros_sb[:, 0:128], zeros_sb[:], start=True, stop=False
                )
            ot_state[(qc, j)] = (ot, lt)

        def pair_groups(qc, j, groups, OT):
            ot, lt = ot_state[(qc, j)]
            q_sl = slice(qc * 512, (qc + 1) * 512)
            hA, hB = 2 * j, 2 * j + 1
            stage = "stage" in ablate
            for g in groups:
                sA = sp.tile([128, 2, 512], f32, tag="stA")
                sB = sp.tile([128, 2, 512], f32, tag="stB")
                for u in range(2):
                    kt = 2 * g + u
                    k_sl = slice(kt * 128, (kt + 1) * 128)
                    nc.tensor.matmul(
                        sA[:, u, :], KT[0:64, j, k_sl], QT[0:64, j, q_sl],
                        start=True, stop=True, tile_position=(0, 0),
                    )
                    nc.tensor.matmul(
                        sB[:, u, :], KT[64:128, j, k_sl], QT[64:128, j, q_sl],
                        start=True, stop=True, tile_position=(64, 0),
                    )
                # Evacuate the two heads' PSUM banks on two engines in
                # parallel: ACT consumes head A directly (exp) while the DVE
                # copies head B to SBUF; head B's exp runs from SBUF off the
                # critical dots->evacuate->dots chain.
                _fn = mybir.ActivationFunctionType.Relu if "relu" in ablate else Exp
                pA = ptp.tile([128, 2, 512], bf16, tag="ptA", name=f"pA{qc}{j}{g}")
                nc.scalar.activation(out=pA[:], in_=sA[:], func=_fn, scale=float(SCALE))
                gB = stgp.tile([128, 2, 512], f32, tag="stgB", name=f"gB{qc}{j}{g}")
                nc.vector.tensor_copy(gB[:], sB[:])
                pB = ptp.tile([128, 2, 512], bf16, tag="ptB", name=f"pB{qc}{j}{g}")
                nc.scalar.activation(out=pB[:], in_=gB[:], func=_fn, scale=float(SCALE))
                for u in range(2):
                    kt = 2 * g + u
                    if "pv" not in ablate:
                        nc.tensor.matmul(
                            ot[0:64, :], V[:, kt, hA * DH : (hA + 1) * DH],
                            pA[:, u, :],
                            start=False, stop=False, tile_position=(0, 0),
                        )
                        nc.tensor.matmul(
                            ot[64:128, :], V[:, kt, hB * DH : (hB + 1) * DH],
                            pB[:, u, :],
                            start=False, stop=False, tile_position=(0, 64),
                        )
                    if "l" not in ablate:
                        nc.tensor.matmul(
                            lt[0:64, :], ones_sb[:, 0:64], pA[:, u, :],
                            start=False, stop=False, tile_position=(0, 0),
                        )
                        nc.tensor.matmul(
                            lt[64:128, :], ones_sb[:, 0:64], pB[:, u, :],
                            start=False, stop=False, tile_position=(0, 64),
                        )

        def pair_end(qc, j, OT):
            if "probe" in ablate:
                ot_state.pop((qc, j))
                if (qc, j) == (0, 0):
                    nc.gpsimd.memset(OT[:], 0.25)
                return
            ot, lt = ot_state.pop((qc, j))
            for t in (ot, lt):  # close the bank's accumulation group (adds zeros)
                nc.tensor.matmul(
                    t[:], zeros_sb[:, 0:128], zeros_sb[:], start=False, stop=True
                )
            rb = rbp.tile([128, 512], f32, tag="rb")
            nc.vector.reciprocal(out=rb[0:64, :], in_=lt[0:64, :])
            nc.vector.reciprocal(out=rb[64:128, :], in_=lt[64:128, :])
            nc.vector.tensor_mul(OT[0:64, j, :], ot[0:64, :], rb[0:64, :])
            nc.vector.tensor_mul(OT[64:128, j, :], ot[64:128, :], rb[64:128, :])

        def emit_y(qc, OT):
            if "probe" in ablate:
                return
            for st in range(4):
                yp = pp.tile([128, 512], f32, tag="proj")
                for dt in range(N_DT):
                    nc.tensor.matmul(
                        yp[:],
                        OT[:, dt, st * 128 : (st + 1) * 128],
                        woutT[:, dt, :],
                        start=(dt == 0),
                        stop=(dt == N_DT - 1),
                    )
                ys = ysp.tile([128, 512], f32, tag="ysb")
                nc.vector.tensor_add(ys[:], yp[:], bias_sb[:])
                nc.sync.dma_start(
                    out=y_out.ap()[qc * 512 + st * 128 : qc * 512 + (st + 1) * 128, :],
                    in_=ys[:],
                )

        OT_tiles = {}
        OT_tiles[0] = otp.tile([128, N_DT, 512], bf16, tag="OT", name="OT0")
        OT_probe_seen = set()

        # --- K/V projection interleaved with the first attention pair ---
        pair_begin(0, 0)
        if "noproj" not in ablate:
            cast_chunk(x_bf, x_full, 0, 512)
            cast_chunk(x_bf, x_full, 512, 512)
        for sc in range(S // 512) if "noproj" not in ablate else []:
            if sc + 2 < S // 512:
                cast_chunk(x_bf, x_full, (sc + 2) * 512, 512)
            xT = xtp.tile([128, N_DT, 512], bf16, tag="xT")
            for dt in range(N_DT):
                nc.sync.dma_start(
                    out=xT[:, dt, :],
                    in_=x_bf.ap()[sc * 512 : (sc + 1) * 512, dt * 128 : (dt + 1) * 128],
                    transpose=True,
                )
            for et in range(N_DT):
                ps = pp.tile([128, 512], f32, tag="proj")
                for dt in range(N_DT):
                    nc.tensor.matmul(
                        ps[:],
                        wqkvT[:, dt, DIM + et * 128 : DIM + (et + 1) * 128],
                        xT[:, dt, :],
                        start=(dt == 0),
                        stop=(dt == N_DT - 1),
                    )
                nc.vector.tensor_copy(KT[:, et, sc * 512 : (sc + 1) * 512], ps[:])
            for a in range(4):
                ps = pp.tile([128, 512], f32, tag="proj")
                for dt in range(N_DT):
                    nc.tensor.matmul(
                        ps[:],
                        xT[:, dt, a * 128 : (a + 1) * 128],
                        wqkvT[:, dt, 2 * DIM : 3 * DIM],
                        start=(dt == 0),
                        stop=(dt == N_DT - 1),
                    )
                nc.vector.tensor_copy(V[:, sc * 4 + a, :], ps[:])
            # attention on pair (qc=0, j=0) for k-groups now available
            pair_groups(0, 0, [2 * sc, 2 * sc + 1], OT_tiles[0])
        if "noproj" in ablate:
            pair_groups(0, 0, list(range(N_KT // 2)), OT_tiles[0])
        pair_end(0, 0, OT_tiles[0])

        # --- remaining pairs ---
        OT_tiles[1] = otp.tile([128, N_DT, 512], bf16, tag="OT", name="OT1")
        if "probe" in ablate:
            nc.gpsimd.memset(OT_tiles[1][:], 0.25)
        for qc, j in [(1, 0), (0, 1), (1, 1), (0, 2), (1, 2), (0, 3), (1, 3)]:
            pair_begin(qc, j)
            pair_groups(qc, j, list(range(N_KT // 2)), OT_tiles[qc])
            pair_end(qc, j, OT_tiles[qc])
            if (qc, j) == (0, 3):
                emit_y(0, OT_tiles[0])
        emit_y(1, OT_tiles[1])

    nc.compile()
    return nc


def _get_nc():
    if "nc" not in _CACHE:
        _CACHE["nc"] = _build_program()
    return _CACHE["nc"]


def sim_time_estimate():
    """CoreSim cost-model span for one core with zero-filled inputs."""
    from concourse.bass_interp import CoreSim

    nc = _get_nc()
    sim = CoreSim(nc, publish_trace=False)
    sim.tensor("x_full")[:] = 0
    sim.tensor("x_q")[:] = 0
    sim.tensor("w_qkv")[:] = 0
    sim.tensor("w_out")[:] = 0
    sim.tensor("b_out")[:] = 0
    sim.simulate()
    return int(sim.time)


def kernel(x, w_qkv, w_out, b_out):
    from concourse.bass_utils import run_bass_kernel_spmd

    nc = _get_nc()
    x = np.asarray(x, dtype=np.float32)
    w_qkv = np.ascontiguousarray(np.asarray(w_qkv, dtype=np.float32))
    w_out = np.ascontiguousarray(np.asarray(w_out, dtype=np.float32))
    b_out = np.ascontiguousarray(np.asarray(b_out, dtype=np.float32)).reshape(1, DIM)

    in_maps = []
    for c in range(N_CORES):
        b = c // 4
        qo = (c % 4) * QLOC
        in_maps.append(
            {
                "x_full": np.ascontiguousarray(x[b]),
                "x_q": np.ascontiguousarray(x[b, qo : qo + QLOC]),
                "w_qkv": w_qkv,
                "w_out": w_out,
                "b_out": b_out,
            }
        )
    res = run_bass_kernel_spmd(nc, in_maps, list(range(N_CORES)))
    y = np.empty((B, S, DIM), dtype=np.float32)
    for c in range(N_CORES):
        b = c // 4
        qo = (c % 4) * QLOC
        y[b, qo : qo + QLOC] = res.results[c]["y"]
    return y



# revision 22
# speedup vs baseline: 1.3461x; 1.3461x over previous
# Multi-head attention (B=2, S=4096, D=512, H=8) on 8 Trainium2 NeuronCores.
#
# Sharding: core c handles batch b=c//4 and query rows [(c%4)*1024, (c%4+1)*1024).
# Each core computes K/V projections for its full batch element (duplicated
# across the 4 cores sharing a batch; avoids all cross-core communication),
# Q for its own slice, full 8-head attention for its query rows, and the
# output projection for its rows. The full output is the disjoint
# concatenation of the 8 per-core results.
#
# On-core dataflow (matmuls bf16 with fp32 PSUM accumulation):
#   x,W --HWDGE load--> SBUF f32 --Pool/DVE cast--> SBUF bf16
#     --PE transpose (identity built via iota)--> x^T, W^T tiles in SBUF
#   (no DRAM round-trip, no DMA-transpose: the PE transposes 128x128 blocks
#   into half-bank bf16 PSUM tiles that share the projection PSUM ring)
#   Q^T = Wq^T-tiles x x^T   K^T = Wk^T-tiles x x^T   V = x^T-tiles x Wv^T
#   V is stored with a ones-column appended per head (65 cols/head), so the
#   PV matmul produces both O^T (64 rows) and the softmax denominator l
#   (row 64) in one instruction -- no separate ones-matmul.
#   per (q-chunk 512, head-pair j), per k-tile kt of 128:
#     S^T[k,q] for BOTH heads -> one 2-bank PSUM tile   (2 row-packed MMs)
#     P^T = exp(S^T * scale): one ACT instruction for both heads, or a
#       Schraudolph bitcast-exp on DVE for a subset of k-tiles (splits the
#       exp throughput across two engines; the softmax denominator uses the
#       same approximated P so the ~3% sawtooth error cancels to 1st order)
#     O^T[0:65,q] += Vx-tile^T x P^T  per head (separate PSUM banks, m=65)
#   PV for k-tile kt is emitted PV_LAG slots behind its scores so the PE
#   never stalls behind the exp engines in its FIFO.
#   pair end: DVE copies O^T+l out of PSUM (frees banks fast), DVE reciprocal
#   of l, broadcast of 1/l across partitions via a small DRAM bounce, DVE
#   mul -> OT (bf16); head B's rows move to partitions 64:128 via SBUF DMA.
#   y = OT-tiles x Wout^T + b  (consumes OT directly as stationary operand)
import sys

if "/opt/trn_rl_repo" not in sys.path:
    sys.path.insert(0, "/opt/trn_rl_repo")

import numpy as np

B = 2
S = 4096
DIM = 512
H = 8
DH = DIM // H
SCALE = DH**-0.5
N_CORES = 8
QLOC = S // 4  # query rows per core
N_KT = S // 128  # k tiles of 128
N_DT = DIM // 128  # feature-dim tiles of 128
PV_LAG = 3  # slots between scores+exp emission and the PV matmul
# Slots (of 32 per pair) whose exp runs on DVE (Schraudolph bitcast exp).
DVE_SLOTS = frozenset({2, 5, 9, 12, 16, 19, 23, 26, 29})
EXP_A = 12102203.161561485  # 2**23 / ln(2)
EXP_B = 1064986823.0  # 127 * 2**23 - 366393 (minimax shift)

_CACHE = {}


def _build_program(reps=1):
    from contextlib import ExitStack

    from concourse import bacc, mybir, tile

    f32 = mybir.dt.float32
    bf16 = mybir.dt.bfloat16
    i32 = mybir.dt.int32
    Exp = mybir.ActivationFunctionType.Exp

    nc = bacc.Bacc("TRN2", target_bir_lowering=False, debug=False)

    x_full = nc.dram_tensor("x_full", [S, DIM], f32, kind="ExternalInput")
    x_q = nc.dram_tensor("x_q", [QLOC, DIM], f32, kind="ExternalInput")
    w_qkv = nc.dram_tensor("w_qkv", [3 * DIM, DIM], f32, kind="ExternalInput")
    w_out = nc.dram_tensor("w_out", [DIM, DIM], f32, kind="ExternalInput")
    b_out = nc.dram_tensor("b_out", [1, DIM], f32, kind="ExternalInput")
    y_out = nc.dram_tensor("y", [QLOC, DIM], f32, kind="ExternalOutput")

    # per-pair 1/l rows bounced through DRAM to broadcast across partitions
    rb_dr = nc.dram_tensor("rb_dr", [8, 2, 512], f32)

    with tile.TileContext(nc) as tc, ExitStack() as ctx:
        if reps > 1:  # benchmarking only: repeat the whole body in a HW loop
            ctx.enter_context(tc.For_i(0, reps, 1))
        consts = ctx.enter_context(tc.tile_pool(name="consts", bufs=1))
        wp = ctx.enter_context(tc.tile_pool(name="wp", bufs=1))
        big = ctx.enter_context(tc.tile_pool(name="big", bufs=1))
        xtp = ctx.enter_context(tc.tile_pool(name="xtp", bufs=2))
        ptp = ctx.enter_context(tc.tile_pool(name="ptp", bufs=6))
        otp = ctx.enter_context(tc.tile_pool(name="otp", bufs=1))
        obp = ctx.enter_context(tc.tile_pool(name="obp", bufs=2))
        rbp = ctx.enter_context(tc.tile_pool(name="rbp", bufs=2))
        ysp = ctx.enter_context(tc.tile_pool(name="ysp", bufs=2))
        castp = ctx.enter_context(tc.tile_pool(name="castp", bufs=3))
        # PSUM: proj/transpose ring 2 banks + sAB 2x2 banks + otA + otB = 8
        pp = ctx.enter_context(tc.tile_pool(name="pp", bufs=2, space="PSUM"))
        sp = ctx.enter_context(tc.tile_pool(name="sp", bufs=2, space="PSUM"))
        op = ctx.enter_context(tc.tile_pool(name="op", bufs=1, space="PSUM"))

        bias_sb = consts.tile([128, DIM], f32, tag="bias")
        nc.gpsimd.dma_start(out=bias_sb[:], in_=b_out.ap().broadcast_to([128, DIM]))

        # identity for PE-mode transposes, built on-chip: (f - p) == 0
        it32 = consts.tile([128, 128], i32, tag="it32")
        nc.gpsimd.iota(it32[:], pattern=[[1, 128]], base=0, channel_multiplier=-1)
        ident = consts.tile([128, 128], bf16, tag="ident")
        nc.gpsimd.tensor_scalar(
            ident[:], it32[:], 0, None, mybir.AluOpType.is_equal
        )

        # --- persistent per-core tensors ---
        KT = big.tile([128, N_DT, S], bf16, tag="KT")  # K^T: part=(e-512)%128, [et, s]
        # V with a ones column appended per head: [k%128, kt, h*65+(dh|64)]
        Vx = big.tile([128, N_KT, H * (DH + 1)], bf16, tag="Vx")
        QT = big.tile([128, N_DT, QLOC], bf16, tag="QT")  # Q^T: part=e%128, [et, q]
        wqkvT = wp.tile([128, N_DT, 3 * DIM], bf16, tag="wqkvT")
        woutT = wp.tile([128, N_DT, DIM], bf16, tag="woutT")

        for h in range(H):
            nc.gpsimd.memset(Vx[:, :, h * 65 + 64 : h * 65 + 65], 1.0)

        # --- load + cast (f32->bf16) a 512-row chunk into SBUF ---
        cast_n = [0]

        def load_cast(src_dram, r0, q=None, eng=None):
            cast_n[0] += 1
            xf = castp.tile([128, 4, DIM], f32, tag="castf", name=f"cf{cast_n[0]}")
            (q or nc.sync).dma_start(
                out=xf[:],
                in_=src_dram.ap()[r0 : r0 + 512, :].rearrange(
                    "(a p) d -> p a d", p=128
                ),
            )
            xb = castp.tile([128, 4, DIM], bf16, tag="castb", name=f"cb{cast_n[0]}")
            (eng or nc.gpsimd).tensor_copy(xb[:], xf[:])
            return xb

        # --- PE-transpose a cast chunk into a [128, dt, 512] T-layout tile ---
        # dst columns c0+a*128+c get rows r0+a*128+p of the source chunk.
        def transpose_chunk(dst, cb, c0=0):
            for dt2 in range(N_DT // 2):
                tp = pp.tile([128, 2, 512], bf16, tag="proj", name=f"tp{cast_n[0]}{dt2}")
                for u in range(2):
                    dt = 2 * dt2 + u
                    for a in range(4):
                        nc.tensor.transpose(
                            tp[:, u, a * 128 : (a + 1) * 128],
                            cb[:, a, dt * 128 : (dt + 1) * 128],
                            ident[:],
                        )
                for u in range(2):
                    dt = 2 * dt2 + u
                    nc.vector.tensor_copy(
                        dst[:, dt, c0 : c0 + 512], tp[:, u, :]
                    )

        def qproj(qc, xqT):
            for et in range(N_DT):
                ps = pp.tile([128, 512], f32, tag="proj")
                for dt in range(N_DT):
                    nc.tensor.matmul(
                        ps[:],
                        wqkvT[:, dt, et * 128 : (et + 1) * 128],
                        xqT[:, dt, :],
                        start=(dt == 0),
                        stop=(dt == N_DT - 1),
                    )
                nc.vector.tensor_copy(QT[:, et, qc * 512 : (qc + 1) * 512], ps[:])

        # --- attention machinery ---
        pair_state = {}

        def pair_begin(qc, j):
            otA = op.tile([128, 512], f32, tag="otA", name=f"otA{qc}{j}")
            otB = op.tile([128, 512], f32, tag="otB", name=f"otB{qc}{j}")
            pair_state[(qc, j)] = [otA, otB, []]

        def emit_pv(qc, j, slot, pH):
            # slot = 2*g + hb: head hb of pair j, k-tiles {2g, 2g+1}.
            # One fp8 DoubleRow matmul contracts both k-tiles and emits
            # O^T (rows 0:64) plus the softmax denominator (row 64).
            otA, otB, _ = pair_state[(qc, j)]
            g, hb = slot // 2, slot % 2
            h = 2 * j + hb
            for u in range(2):
                nc.tensor.matmul(
                    (otB if hb else otA)[0:65, :],
                    Vx[:, 2 * g + u, h * 65 : h * 65 + 65],
                    pH[:, u, :],
                    start=(g == 0 and u == 0),
                    stop=(g == N_KT // 2 - 1 and u == 1),
                )

        def pair_slots(qc, j, slots, dve_slots=frozenset()):
            st = pair_state[(qc, j)]
            q_sl = slice(qc * 512, (qc + 1) * 512)
            for slot in slots:
                g, hb = slot // 2, slot % 2
                r_sl = slice(64, 128) if hb else slice(0, 64)
                sH = sp.tile([128, 2, 512], f32, tag="sAB", name=f"s{qc}{j}{slot}")
                for u in range(2):
                    kt = 2 * g + u
                    nc.tensor.matmul(
                        sH[:, u, :],
                        KT[r_sl, j, kt * 128 : (kt + 1) * 128],
                        QT[r_sl, j, q_sl],
                        start=True, stop=True,
                        tile_position=(64 * hb, 0),
                    )
                pH = ptp.tile([128, 2, 512], bf16, tag="pt", name=f"p{qc}{j}{slot}")
                if slot in dve_slots:
                    ti = ptp.tile(
                        [128, 2, 512], i32, tag="ti", bufs=3, name=f"ti{qc}{j}{slot}"
                    )
                    nc.vector.tensor_scalar(
                        ti[:], sH[:], float(SCALE * EXP_A), EXP_B,
                        mybir.AluOpType.mult, mybir.AluOpType.add,
                    )
                    nc.vector.tensor_copy(pH[:], ti[:].bitcast(f32))
                else:
                    nc.scalar.activation(
                        out=pH[:], in_=sH[:], func=Exp, scale=float(SCALE)
                    )
                st[2].append((slot, pH))
                while len(st[2]) > PV_LAG:
                    pslot, ppH = st[2].pop(0)
                    emit_pv(qc, j, pslot, ppH)

        def pair_end(qc, j, OT):
            st = pair_state[(qc, j)]
            for pkt, ppAB in st[2]:
                emit_pv(qc, j, pkt, ppAB)
            otA, otB, _ = pair_state.pop((qc, j))
            # evacuate O^T + l from PSUM promptly so next pair's PVs can start
            obA = obp.tile([128, 512], f32, tag="obA", name=f"obA{qc}{j}")
            nc.vector.tensor_copy(obA[0:65, :], otA[0:65, :])
            obB = obp.tile([128, 512], f32, tag="obB", name=f"obB{qc}{j}")
            nc.vector.tensor_copy(obB[0:65, :], otB[0:65, :])
            # 1/l rows; broadcast across partitions via a DRAM bounce
            rbs = rbp.tile([128, 2, 512], f32, tag="rbs", name=f"rbs{qc}{j}")
            nc.vector.reciprocal(out=rbs[64:65, 0, :], in_=obA[64:65, :])
            nc.vector.reciprocal(out=rbs[64:65, 1, :], in_=obB[64:65, :])
            pi = qc * 4 + j
            nc.sync.dma_start(out=rb_dr.ap()[pi, :, :], in_=rbs[64:65, :, :])
            rbb = rbp.tile([128, 512], f32, tag="rbb", name=f"rbb{qc}{j}")
            nc.sync.dma_start(
                out=rbb[0:64, :], in_=rb_dr.ap()[pi, 0:1, :].broadcast_to([64, 512])
            )
            nc.sync.dma_start(
                out=rbb[64:128, :], in_=rb_dr.ap()[pi, 1:2, :].broadcast_to([64, 512])
            )
            # head B rows must land at partitions 64:128 of OT: move via DMA
            obBh = obp.tile([128, 512], f32, tag="obBh", name=f"obBh{qc}{j}")
            nc.sync.dma_start(out=obBh[64:128, :], in_=obB[0:64, :])
            nc.vector.tensor_mul(OT[0:64, j, :], obA[0:64, :], rbb[0:64, :])
            nc.vector.tensor_mul(OT[64:128, j, :], obBh[64:128, :], rbb[64:128, :])

        def emit_y(qc, OT):
            for st in range(4):
                yp = pp.tile([128, 512], f32, tag="proj")
                for dt in range(N_DT):
                    nc.tensor.matmul(
                        yp[:],
                        OT[:, dt, st * 128 : (st + 1) * 128],
                        woutT[:, dt, :],
                        start=(dt == 0),
                        stop=(dt == N_DT - 1),
                    )
                ys = ysp.tile([128, 512], f32, tag="ysb")
                nc.vector.tensor_add(ys[:], yp[:], bias_sb[:])
                nc.sync.dma_start(
                    out=y_out.ap()[qc * 512 + st * 128 : qc * 512 + (st + 1) * 128, :],
                    in_=ys[:],
                )

        OT_tiles = {}
        OT_tiles[0] = otp.tile([128, N_DT, 512], bf16, tag="OT0", name="OT0")
        OT_tiles[1] = otp.tile([128, N_DT, 512], bf16, tag="OT1", name="OT1")

        # --- startup: weights + Q projection, x chunks prefetching ---
        cbw0 = load_cast(w_qkv, 0, q=nc.sync)
        cbx = {0: load_cast(x_full, 0, q=nc.scalar)}
        transpose_chunk(wqkvT, cbw0, 0)
        cbq0 = load_cast(x_q, 0, q=nc.sync, eng=nc.vector)
        cbx[1] = load_cast(x_full, 512, q=nc.scalar)
        xqT0 = xtp.tile([128, N_DT, 512], bf16, tag="xqT")
        transpose_chunk(xqT0, cbq0)
        qproj(0, xqT0)
        cbw1 = load_cast(w_qkv, 512, q=nc.sync)
        transpose_chunk(wqkvT, cbw1, 512)
        cbq1 = load_cast(x_q, 512, q=nc.sync, eng=nc.vector)
        xqT1 = xtp.tile([128, N_DT, 512], bf16, tag="xqT")
        transpose_chunk(xqT1, cbq1)
        qproj(1, xqT1)
        cbw2 = load_cast(w_qkv, 1024, q=nc.sync)
        transpose_chunk(wqkvT, cbw2, 1024)
        cbwo = load_cast(w_out, 0, q=nc.sync, eng=nc.vector)
        transpose_chunk(woutT, cbwo)
        xT = {0: xtp.tile([128, N_DT, 512], bf16, tag="xT", name="xT0")}
        transpose_chunk(xT[0], cbx[0])

        # --- K/V projection interleaved with the first attention pair ---
        pair_begin(0, 0)
        for sc in range(S // 512):
            if sc + 2 < S // 512:
                cbx[sc + 2] = load_cast(
                    x_full, (sc + 2) * 512, q=(nc.scalar if sc % 2 else nc.sync)
                )
            for et in range(N_DT):
                ps = pp.tile([128, 512], f32, tag="proj")
                for dt in range(N_DT):
                    nc.tensor.matmul(
                        ps[:],
                        wqkvT[:, dt, DIM + et * 128 : DIM + (et + 1) * 128],
                        xT[sc][:, dt, :],
                        start=(dt == 0),
                        stop=(dt == N_DT - 1),
                    )
                nc.vector.tensor_copy(KT[:, et, sc * 512 : (sc + 1) * 512], ps[:])
            for a in range(4):
                ps = pp.tile([128, 512], f32, tag="proj")
                for dt in range(N_DT):
                    nc.tensor.matmul(
                        ps[:],
                        xT[sc][:, dt, a * 128 : (a + 1) * 128],
                        wqkvT[:, dt, 2 * DIM : 3 * DIM],
                        start=(dt == 0),
                        stop=(dt == N_DT - 1),
                    )
                # scatter the 8 head slices into Vx (65-wide per head)
                nc.vector.tensor_copy(
                    Vx[:, sc * 4 + a, :]
                    .rearrange("p (h c) -> p h c", h=H)[:, :, 0:DH],
                    ps[:].rearrange("p (h c) -> p h c", h=H),
                )
            if sc + 1 < S // 512:
                xT[sc + 1] = xtp.tile(
                    [128, N_DT, 512], bf16, tag="xT", name=f"xT{sc + 1}"
                )
                transpose_chunk(xT[sc + 1], cbx[sc + 1])
            # attention on pair (qc=0, j=0) for the 4 k-tiles just produced
            pair_slots(0, 0, [4 * sc, 4 * sc + 1, 4 * sc + 2, 4 * sc + 3])
        pair_end(0, 0, OT_tiles[0])

        # --- remaining pairs (exp split across ACT + DVE) ---
        for qc, j in [(1, 0), (0, 1), (1, 1), (0, 2), (1, 2), (0, 3), (1, 3)]:
            pair_begin(qc, j)
            pair_slots(qc, j, list(range(N_KT)), dve_slots=DVE_SLOTS)
            pair_end(qc, j, OT_tiles[qc])
            if (qc, j) == (0, 3):
                emit_y(0, OT_tiles[0])
        emit_y(1, OT_tiles[1])

    nc.compile()
    return nc


def _get_nc():
    if "nc" not in _CACHE:
        _CACHE["nc"] = _build_program()
    return _CACHE["nc"]


def sim_time_estimate():
    """CoreSim cost-model span for one core with zero-filled inputs."""
    from concourse.bass_interp import CoreSim

    nc = _get_nc()
    sim = CoreSim(nc, publish_trace=False)
    sim.tensor("x_full")[:] = 0
    sim.tensor("x_q")[:] = 0
    sim.tensor("w_qkv")[:] = 0
    sim.tensor("w_out")[:] = 0
    sim.tensor("b_out")[:] = 0
    sim.simulate()
    return int(sim.time)


def kernel(x, w_qkv, w_out, b_out):
    from concourse.bass_utils import run_bass_kernel_spmd

    nc = _get_nc()
    x = np.asarray(x, dtype=np.float32)
    w_qkv = np.ascontiguousarray(np.asarray(w_qkv, dtype=np.float32))
    w_out = np.ascontiguousarray(np.asarray(w_out, dtype=np.float32))
    b_out = np.ascontiguousarray(np.asarray(b_out, dtype=np.float32)).reshape(1, DIM)

    in_maps = []
    for c in range(N_CORES):
        b = c // 4
        qo = (c % 4) * QLOC
        in_maps.append(
            {
                "x_full": np.ascontiguousarray(x[b]),
                "x_q": np.ascontiguousarray(x[b, qo : qo + QLOC]),
                "w_qkv": w_qkv,
                "w_out": w_out,
                "b_out": b_out,
            }
        )
    res = run_bass_kernel_spmd(nc, in_maps, list(range(N_CORES)))
    y = np.empty((B, S, DIM), dtype=np.float32)
    for c in range(N_CORES):
        b = c // 4
        qo = (c % 4) * QLOC
        y[b, qo : qo + QLOC] = res.results[c]["y"]
    return y
